# revision 1
# baseline (speedup 1.0000x reference)
"""Local sliding-window attention block (MQA + partial RoPE) on 8 TRN2 cores.

Sharding: 2 batches x 4 sequence chunks of 512 queries each. Each core
computes q/k/v projections for its chunk (keys include a 512-token halo),
windowed attention (window=512, causal), and the o-projection for its own
query rows — so the host-side unshard is a pure concatenation.

On-chip layout: everything transposed (feature dim on partitions).
  xT[d, pos]  ->  Q^T[dh, q] / K^T[dh, k] (RoPE'd)  ->  S^T[k, q]
  -> exp -> P^T[k, q] (bf16, multiplicative 0/1 masks)
  -> O^T[dv, q] = V.T-matmul  -> normalized by softmax denominators
     (partition_all_reduce on GPSIMD)  -> used directly as lhsT of o-proj.
All matmuls bf16 inputs, fp32 PSUM accumulation.
"""

import numpy as np
import ml_dtypes

BF16 = ml_dtypes.bfloat16

B, L, D = 2, 2048, 2048
H, HD = 16, 128
ROPE_DIMS, HALF = 64, 32
WINDOW = 512
ROPE_BASE = 10000.0
SCALE = HD ** -0.5

CHUNK = 512            # queries per core
NK = 1024              # keys (incl. halo) per core
NQT = CHUNK // 128     # 4 local query tiles
NKT = NK // 128        # 8 local key tiles
NSIG = 5               # key tiles in window per query tile
NDT = D // 128         # 16 contraction tiles over embedding dim

_PROGRAM = None


def _rope(nc, mybir, pool_tmp, out_bf, ps, cos2, sin2m, fp32):
    """out[0:64] = rotary(ps[0:64]); out[64:128] = ps[64:128]. ps fp32, out bf16.

    cos2 is [64, n] with rows [0:32]==[32:64]==cos(theta); sin2m has rows
    [0:32]==-sin(theta), [32:64]==+sin(theta). Engines can't read across
    partitions, so the half-swap (x2 into rows 0:32, x1 into rows 32:64)
    goes through two partition-shifting DMAs; then
      out[0:64] = ps[0:64]*cos2 + swapped*sin2m
    is partition-aligned elementwise math.
    """
    n = cos2.shape[-1]
    sb64 = pool_tmp.tile([ROPE_DIMS, n], fp32, tag="rope_sb64")
    nc.scalar.copy(sb64, ps[0:ROPE_DIMS])
    ss = pool_tmp.tile([ROPE_DIMS, n], fp32, tag="rope_ss")
    nc.sync.dma_start(out=ss[0:HALF], in_=sb64[HALF:ROPE_DIMS])
    nc.sync.dma_start(out=ss[HALF:ROPE_DIMS], in_=sb64[0:HALF])
    mcos = pool_tmp.tile([ROPE_DIMS, n], fp32, tag="rope_mcos")
    nc.vector.tensor_mul(mcos, ps[0:ROPE_DIMS], cos2)
    nc.vector.tensor_mul(ss, ss, sin2m)
    nc.vector.tensor_add(out_bf[0:ROPE_DIMS], mcos, ss)
    nc.scalar.copy(out_bf[ROPE_DIMS:HD], ps[ROPE_DIMS:HD])


def _build_program():
    from contextlib import ExitStack
    import concourse.bass as bass
    import concourse.mybir as mybir
    import concourse.tile as tile
    import concourse.bass_isa as bass_isa
    from concourse import bacc

    fp32 = mybir.dt.float32
    bf16 = mybir.dt.bfloat16
    AF = mybir.ActivationFunctionType

    nc = bacc.Bacc(None, target_bir_lowering=False)

    xT_d = nc.dram_tensor("xT", [D, NK], bf16, kind="ExternalInput")
    wq_d = nc.dram_tensor("Wq", [D, D], bf16, kind="ExternalInput")
    wk_d = nc.dram_tensor("Wk", [D, HD], bf16, kind="ExternalInput")
    wv_d = nc.dram_tensor("Wv", [D, HD], bf16, kind="ExternalInput")
    wo_d = nc.dram_tensor("Wo", [D, D], bf16, kind="ExternalInput")
    bo_d = nc.dram_tensor("bo", [1, D], fp32, kind="ExternalInput")
    cos_d = nc.dram_tensor("cosT", [ROPE_DIMS, NK], fp32, kind="ExternalInput")
    sin_d = nc.dram_tensor("sinT", [ROPE_DIMS, NK], fp32, kind="ExternalInput")
    msk_d = nc.dram_tensor("masks", [NQT, NSIG, 128, 128], bf16, kind="ExternalInput")
    out_d = nc.dram_tensor("out", [CHUNK, D], fp32, kind="ExternalOutput")

    with tile.TileContext(nc) as tc, ExitStack() as ctx:
        p_const = ctx.enter_context(tc.tile_pool(name="const", bufs=1))
        p_xt = ctx.enter_context(tc.tile_pool(name="xt", bufs=1))
        p_kv = ctx.enter_context(tc.tile_pool(name="kv", bufs=1))
        p_wq = ctx.enter_context(tc.tile_pool(name="wq", bufs=3))
        p_qt = ctx.enter_context(tc.tile_pool(name="qt", bufs=3))
        p_es = ctx.enter_context(tc.tile_pool(name="es", bufs=6))
        p_red = ctx.enter_context(tc.tile_pool(name="red", bufs=4))
        p_tmp = ctx.enter_context(tc.tile_pool(name="tmp", bufs=4))
        p_otn = ctx.enter_context(tc.tile_pool(name="otn", bufs=1))
        p_wo = ctx.enter_context(tc.tile_pool(name="wo", bufs=3))
        p_ob = ctx.enter_context(tc.tile_pool(name="ob", bufs=4))

        # ---- persistent loads ----
        xview = xT_d[:].rearrange("(n p) m -> n p m", p=128)
        xt = []
        for i in range(NDT):
            t_ = p_xt.tile([128, NK], bf16, tag=f"xt{i}")
            nc.sync.dma_start(out=t_, in_=xview[i])
            xt.append(t_)

        wk_sb = p_const.tile([128, NDT, HD], bf16, tag="wk")
        nc.sync.dma_start(out=wk_sb, in_=wk_d[:].rearrange("(n p) m -> p n m", p=128))
        wv_sb = p_const.tile([128, NDT, HD], bf16, tag="wv")
        nc.sync.dma_start(out=wv_sb, in_=wv_d[:].rearrange("(n p) m -> p n m", p=128))

        cos_sb = p_const.tile([ROPE_DIMS, NK], fp32, tag="cos")
        nc.sync.dma_start(out=cos_sb, in_=cos_d[:])
        sin_sb = p_const.tile([ROPE_DIMS, NK], fp32, tag="sin")
        nc.sync.dma_start(out=sin_sb, in_=sin_d[:])

        msk_sb = p_const.tile([128, NQT, NSIG, 128], bf16, tag="msk")
        nc.sync.dma_start(out=msk_sb, in_=msk_d[:].rearrange("t s k q -> k t s q"))

        bias_sb = p_const.tile([128, D], fp32, tag="bias")
        nc.sync.dma_start(
            out=bias_sb, in_=bass.AP(tensor=bo_d, offset=0, ap=[[0, 128], [1, D]])
        )

        # ---- K^T (RoPE'd) and V projections ----
        kt = p_kv.tile([128, NK], bf16, tag="kt")
        v_sb = []
        for s in range(NKT):
            t_ = p_kv.tile([128, HD], bf16, tag=f"v{s}")
            v_sb.append(t_)

        with tc.tile_pool(name="ps_kv", bufs=2, space=bass.MemorySpace.PSUM) as ps_kv:
            for nh in range(NK // 512):
                ps = ps_kv.tile([128, 512], fp32, tag="ps_kv")
                cols = slice(nh * 512, (nh + 1) * 512)
                for dt in range(NDT):
                    nc.tensor.matmul(
                        ps, wk_sb[:, dt, :], xt[dt][:, cols],
                        start=(dt == 0), stop=(dt == NDT - 1),
                    )
                _rope(nc, mybir, p_tmp, kt[:, cols], ps,
                      cos_sb[:, cols], sin_sb[:, cols], fp32)

            for s in range(NKT):
                psv = ps_kv.tile([128, HD], fp32, tag="ps_v")
                cols = slice(s * 128, (s + 1) * 128)
                for dt in range(NDT):
                    nc.tensor.matmul(
                        psv, xt[dt][:, cols], wv_sb[:, dt, :],
                        start=(dt == 0), stop=(dt == NDT - 1),
                    )
                nc.scalar.copy(v_sb[s], psv)

        # ---- per-head attention ----
        otn = []
        for h in range(H):
            t_ = p_otn.tile([128, CHUNK], bf16, tag=f"otn{h}")
            otn.append(t_)

        wqview = wq_d[:].rearrange("(n p) m -> p n m", p=128)
        with (
            tc.tile_pool(name="ps_q", bufs=2, space=bass.MemorySpace.PSUM) as ps_qp,
            tc.tile_pool(name="ps_s", bufs=2, space=bass.MemorySpace.PSUM) as ps_sp,
            tc.tile_pool(name="ps_o", bufs=2, space=bass.MemorySpace.PSUM) as ps_op,
        ):
            for h in range(H):
                wq_h = p_wq.tile([128, NDT, 128], bf16, tag="wq")
                nc.sync.dma_start(out=wq_h, in_=wqview[:, :, h * 128:(h + 1) * 128])
                psq = ps_qp.tile([128, CHUNK], fp32, tag="ps_q")
                for dt in range(NDT):
                    nc.tensor.matmul(
                        psq, wq_h[:, dt, :], xt[dt][:, CHUNK:NK],
                        start=(dt == 0), stop=(dt == NDT - 1),
                    )
                qt = p_qt.tile([128, CHUNK], bf16, tag="qt")
                _rope(nc, mybir, p_tmp, qt, psq,
                      cos_sb[:, CHUNK:NK], sin_sb[:, CHUNK:NK], fp32)

                otp = ps_op.tile([128, CHUNK], fp32, tag="ps_o")
                recip = p_red.tile([128, NQT, 128], fp32, tag="recip")
                for t in range(NQT):
                    pss = ps_sp.tile([128, NSIG, 128], fp32, tag="ps_s")
                    qsl = qt[:, t * 128:(t + 1) * 128]
                    for sig in range(NSIG):
                        s = t + sig
                        nc.tensor.matmul(
                            pss[:, sig, :], kt[:, s * 128:(s + 1) * 128], qsl,
                            start=True, stop=True,
                        )
                    es = p_es.tile([128, NSIG, 128], bf16, tag="es")
                    nc.scalar.activation(es, pss, AF.Exp, scale=SCALE)
                    nc.vector.tensor_mul(es, es, msk_sb[:, t, :, :])
                    red = p_red.tile([128, 128], fp32, tag="red")
                    nc.vector.reduce_sum(
                        out=red, in_=es.rearrange("p s q -> p q s"),
                        axis=mybir.AxisListType.X,
                    )
                    nc.gpsimd.partition_all_reduce(
                        recip[:, t, :], red, channels=128,
                        reduce_op=bass_isa.ReduceOp.add,
                    )
                    for sig in range(NSIG):
                        nc.tensor.matmul(
                            otp[:, t * 128:(t + 1) * 128],
                            v_sb[t + sig], es[:, sig, :],
                            start=(sig == 0), stop=(sig == NSIG - 1),
                        )
                rview = recip.rearrange("p t q -> p (t q)")
                nc.vector.reciprocal(rview, rview)
                nc.vector.tensor_mul(otn[h], otp, rview)

        # ---- o-projection + bias ----
        woview = wo_d[:].rearrange("(h p) m -> p h m", p=128)
        with tc.tile_pool(name="ps_out", bufs=4, space=bass.MemorySpace.PSUM) as ps_outp:
            for n in range(D // 512):
                wo_n = p_wo.tile([128, H, 512], bf16, tag="wo")
                nc.sync.dma_start(out=wo_n, in_=woview[:, :, n * 512:(n + 1) * 512])
                for t in range(NQT):
                    pso = ps_outp.tile([128, 512], fp32, tag="ps_out")
                    for h in range(H):
                        nc.tensor.matmul(
                            pso, otn[h][:, t * 128:(t + 1) * 128], wo_n[:, h, :],
                            start=(h == 0), stop=(h == H - 1),
                        )
                    ob = p_ob.tile([128, 512], fp32, tag="ob")
                    nc.vector.tensor_add(ob, pso, bias_sb[:, n * 512:(n + 1) * 512])
                    nc.sync.dma_start(
                        out=out_d[t * 128:(t + 1) * 128, n * 512:(n + 1) * 512],
                        in_=ob,
                    )

    nc.compile()
    return nc


def _get_program():
    global _PROGRAM
    if _PROGRAM is None:
        _PROGRAM = _build_program()
    return _PROGRAM


def _make_in_maps(x, Wq, Wk, Wv, Wo, bo):
    Wq_b = np.ascontiguousarray(Wq.astype(BF16))
    Wk_b = np.ascontiguousarray(Wk.astype(BF16))
    Wv_b = np.ascontiguousarray(Wv.astype(BF16))
    Wo_b = np.ascontiguousarray(Wo.astype(BF16))
    bo_f = np.ascontiguousarray(bo.astype(np.float32).reshape(1, D))

    inv_freq = np.exp(
        -np.log(np.float32(ROPE_BASE))
        * (np.arange(0, ROPE_DIMS, 2, dtype=np.float32) / np.float32(ROPE_DIMS))
    ).astype(np.float32)

    in_maps = []
    for c in range(8):
        b, g = divmod(c, 4)
        k_start = 512 * g - 512
        xs = np.zeros((NK, D), np.float32)
        lo = max(0, k_start)
        xs[lo - k_start:] = x[b, lo:k_start + NK]
        xT = np.ascontiguousarray(xs.T).astype(BF16)

        pos = (k_start + np.arange(NK)).astype(np.float32)
        theta = pos[None, :] * inv_freq[:, None]          # [32, NK]
        cos2 = np.ascontiguousarray(
            np.concatenate([np.cos(theta)] * 2, axis=0).astype(np.float32))
        sin2 = np.ascontiguousarray(
            np.concatenate([-np.sin(theta), np.sin(theta)], axis=0).astype(np.float32))

        m = np.zeros((NQT, NSIG, 128, 128), np.float32)
        for t in range(NQT):
            Tg = NQT * g + t
            for sig in range(NSIG):
                S = Tg - 4 + sig
                if S < 0:
                    continue
                i = (128 * Tg + np.arange(128))[None, :]   # queries (cols)
                j = (128 * S + np.arange(128))[:, None]    # keys (rows)
                m[t, sig] = (((i - j) >= 0) & ((i - j) < WINDOW)).astype(np.float32)
        masks = np.ascontiguousarray(m.astype(BF16))

        in_maps.append({
            "xT": xT, "Wq": Wq_b, "Wk": Wk_b, "Wv": Wv_b, "Wo": Wo_b,
            "bo": bo_f, "cosT": cos2, "sinT": sin2, "masks": masks,
        })
    return in_maps


def _unshard(results):
    out = np.zeros((B, L, D), np.float32)
    for c in range(8):
        b, g = divmod(c, 4)
        out[b, CHUNK * g:CHUNK * (g + 1)] = results[c]["out"]
    return out


def kernel(x, Wq, Wk, Wv, Wo, bo):
    from concourse.bass_utils import run_bass_kernel_spmd

    nc = _get_program()
    in_maps = _make_in_maps(x, Wq, Wk, Wv, Wo, bo)
    res = run_bass_kernel_spmd(nc, in_maps, core_ids=list(range(8)))
    return _unshard(res.results)



# revision 4
# speedup vs baseline: 1.4756x; 1.4756x over previous
"""Local sliding-window attention block (MQA + partial RoPE) on 8 TRN2 cores.

Sharding: 2 batches x 4 sequence chunks of 512 queries each. Each core
computes q/k/v projections for its chunk (keys include a 512-token halo),
windowed attention (window=512, causal), and the o-projection for its own
query rows — the host-side unshard is a pure concatenation.

v2 layout/engine plan:
  - All weights pre-permuted on the host so every DMA is contiguous per
    partition (4-16 KB packets instead of 256 B).
  - S^T tiles computed key-tile-major: one matmul per key tile s with the
    contiguous valid query range as the moving operand (8 wide matmuls per
    head instead of 20 narrow ones).
  - Masks are two universal 128x128 tiles (far edge / diagonal); key-halo
    zero-padding leaks exp(0)=1 terms into the softmax denominator, which
    are removed by a host-precomputed per-query leak count (cvec).
  - AV uses es (P^T tile) as the stationary operand and V with an appended
    ones-column as the moving operand: out[q, 0:128] = attention numerator,
    out[q, 128] = softmax denominator — no GPSIMD partition reduce, no
    strided sig-reduce, and the reciprocal shrinks to [128, 4] per head
    (reciprocal_approx_fast).
  - Normalized O tiles are transposed back to [hd, q] with PE transpose-mode
    matmuls to feed the o-projection as stationary operands.
  - Head loop is software-pipelined: Q-proj of head h+1 and the O-transposes
    of head h-1 are interleaved so the tensor queue never waits on the
    RoPE/softmax chains.
"""

import numpy as np
import ml_dtypes

BF16 = ml_dtypes.bfloat16
F16 = np.float16

B, L, D = 2, 2048, 2048
H, HD = 16, 128
ROPE_DIMS, HALF = 64, 32
WINDOW = 512
ROPE_BASE = 10000.0
SCALE = HD ** -0.5

CHUNK = 512            # queries per core
NK = 1024              # keys (incl. halo) per core
NQT = CHUNK // 128     # 4 local query tiles
NKT = NK // 128        # 8 local key tiles
NDT = D // 128         # 16 contraction tiles over embedding dim
NOG = D // 512         # 4 o-proj column groups

_PROGRAM = None


def _build_program():
    from contextlib import ExitStack
    import concourse.bass as bass
    import concourse.mybir as mybir
    import concourse.tile as tile
    from concourse import bacc

    fp32 = mybir.dt.float32
    bf16 = mybir.dt.bfloat16
    f16 = mybir.dt.float16
    AF = mybir.ActivationFunctionType

    nc = bacc.Bacc(None, target_bir_lowering=False)

    xT_d = nc.dram_tensor("xT", [D, NK], bf16, kind="ExternalInput")
    wq_d = nc.dram_tensor("WqP", [128, H * NDT * 128], bf16, kind="ExternalInput")
    wk_d = nc.dram_tensor("WkP", [128, NDT * HD], bf16, kind="ExternalInput")
    wv_d = nc.dram_tensor("WvP", [128, NDT * HD], bf16, kind="ExternalInput")
    wo_d = nc.dram_tensor("WoP", [128, NOG * H * 512], bf16, kind="ExternalInput")
    bo_d = nc.dram_tensor("bo", [1, D], fp32, kind="ExternalInput")
    cos_d = nc.dram_tensor("cosT", [ROPE_DIMS, NK], f16, kind="ExternalInput")
    sin_d = nc.dram_tensor("sinT", [ROPE_DIMS, NK], f16, kind="ExternalInput")
    msk_d = nc.dram_tensor("maskT", [128, 256], bf16, kind="ExternalInput")
    cvec_d = nc.dram_tensor("cvec", [128, NQT], fp32, kind="ExternalInput")
    eye_d = nc.dram_tensor("eye", [128, 128], bf16, kind="ExternalInput")
    out_d = nc.dram_tensor("out", [CHUNK, D], fp32, kind="ExternalOutput")

    def qrange(s):
        # query tiles t covered by key tile s: t in [max(0,s-4), min(3,s)]
        tlo, thi = max(0, s - 4), min(NQT - 1, s)
        return tlo * 128, (thi + 1) * 128

    with tile.TileContext(nc) as tc, ExitStack() as ctx:
        p_const = ctx.enter_context(tc.tile_pool(name="const", bufs=1))
        p_xt = ctx.enter_context(tc.tile_pool(name="xt", bufs=1))
        p_kv = ctx.enter_context(tc.tile_pool(name="kv", bufs=1))
        p_wq = ctx.enter_context(tc.tile_pool(name="wq", bufs=3))
        p_qt = ctx.enter_context(tc.tile_pool(name="qt", bufs=3))
        p_es = ctx.enter_context(tc.tile_pool(name="es", bufs=12))
        p_tmp = ctx.enter_context(tc.tile_pool(name="tmp", bufs=3))
        p_otn = ctx.enter_context(tc.tile_pool(name="otn", bufs=1))
        p_od = ctx.enter_context(tc.tile_pool(name="od", bufs=3))
        p_wo = ctx.enter_context(tc.tile_pool(name="wo", bufs=2))
        p_ob = ctx.enter_context(tc.tile_pool(name="ob", bufs=4))

        def _rope(out_bf, ps, cos2, sin2m):
            """out[0:64] = rotary(ps[0:64]); out[64:128] = ps[64:128].

            cos2 rows [0:32]==[32:64]==cos(theta); sin2m rows [0:32]==-sin,
            [32:64]==+sin. The half-swap goes through two partition-shifting
            SBUF->SBUF DMAs; everything else is partition-aligned math.
            """
            n = cos2.shape[-1]
            sb64 = p_tmp.tile([ROPE_DIMS, n], f16, tag="rope_sb64")
            nc.scalar.copy(sb64, ps[0:ROPE_DIMS])
            ss = p_tmp.tile([ROPE_DIMS, n], f16, tag="rope_ss")
            nc.sync.dma_start(out=ss[0:HALF], in_=sb64[HALF:ROPE_DIMS])
            nc.sync.dma_start(out=ss[HALF:ROPE_DIMS], in_=sb64[0:HALF])
            mcos = p_tmp.tile([ROPE_DIMS, n], f16, tag="rope_mcos")
            nc.vector.tensor_mul(mcos, sb64, cos2)
            nc.vector.tensor_mul(ss, ss, sin2m)
            nc.vector.tensor_add(out_bf[0:ROPE_DIMS], mcos, ss)
            nc.scalar.copy(out_bf[ROPE_DIMS:HD], ps[ROPE_DIMS:HD])

        # ---- persistent loads (all contiguous per partition) ----
        xview = xT_d[:].rearrange("(n p) m -> n p m", p=128)
        xt = []
        for i in range(NDT):
            t_ = p_xt.tile([128, NK], bf16, tag=f"xt{i}")
            nc.sync.dma_start(out=t_, in_=xview[i])
            xt.append(t_)

        wk_sb = p_const.tile([128, NDT, HD], bf16, tag="wk")
        nc.sync.dma_start(
            out=wk_sb, in_=wk_d[:].rearrange("p (n m) -> p n m", n=NDT)
        )
        wv_sb = p_const.tile([128, NDT, HD], bf16, tag="wv")
        nc.sync.dma_start(
            out=wv_sb, in_=wv_d[:].rearrange("p (n m) -> p n m", n=NDT)
        )

        cos_sb = p_const.tile([ROPE_DIMS, NK], f16, tag="cos")
        nc.sync.dma_start(out=cos_sb, in_=cos_d[:])
        sin_sb = p_const.tile([ROPE_DIMS, NK], f16, tag="sin")
        nc.sync.dma_start(out=sin_sb, in_=sin_d[:])

        msk_sb = p_const.tile([128, 256], bf16, tag="msk")
        nc.sync.dma_start(out=msk_sb, in_=msk_d[:])
        cvec_sb = p_const.tile([128, NQT], fp32, tag="cvec")
        nc.sync.dma_start(out=cvec_sb, in_=cvec_d[:])
        eye_sb = p_const.tile([128, 128], bf16, tag="eye")
        nc.sync.dma_start(out=eye_sb, in_=eye_d[:])

        bias_sb = p_const.tile([128, D], fp32, tag="bias")
        nc.sync.dma_start(
            out=bias_sb, in_=bass.AP(tensor=bo_d, offset=0, ap=[[0, 128], [1, D]])
        )

        # ---- K^T (RoPE'd) and V (+ones col) projections ----
        kt = p_kv.tile([128, NK], bf16, tag="kt")
        v_ext = []
        for s in range(NKT):
            t_ = p_kv.tile([128, HD + 1], bf16, tag=f"v{s}")
            nc.vector.memset(t_[:, HD:HD + 1], 1.0)
            v_ext.append(t_)

        with tc.tile_pool(name="ps_kv", bufs=2, space=bass.MemorySpace.PSUM) as ps_kv:
            for nh in range(NK // 512):
                ps = ps_kv.tile([128, 512], fp32, tag="ps_kv")
                cols = slice(nh * 512, (nh + 1) * 512)
                for dt in range(NDT):
                    nc.tensor.matmul(
                        ps, wk_sb[:, dt, :], xt[dt][:, cols],
                        start=(dt == 0), stop=(dt == NDT - 1),
                    )
                _rope(kt[:, cols], ps, cos_sb[:, cols], sin_sb[:, cols])

            for s in range(NKT):
                psv = ps_kv.tile([128, HD], fp32, tag="ps_v")
                cols = slice(s * 128, (s + 1) * 128)
                for dt in range(NDT):
                    nc.tensor.matmul(
                        psv, xt[dt][:, cols], wv_sb[:, dt, :],
                        start=(dt == 0), stop=(dt == NDT - 1),
                    )
                nc.scalar.copy(v_ext[s][:, 0:HD], psv)

        # ---- per-head attention (software-pipelined) ----
        otn = []
        for h in range(H):
            t_ = p_otn.tile([128, CHUNK], bf16, tag=f"otn{h}")
            otn.append(t_)

        wqview = wq_d[:].rearrange("p (h n m) -> p h n m", h=H, n=NDT)
        qt = [None] * H        # rope'd Q^T per head
        esbuf = [None] * H     # 8 es tiles per head
        obf = [None] * H       # [q, t, hd+1] fp32 numerators + denominators
        rcp = [None] * H       # [q, t] reciprocal denominators

        with (
            tc.tile_pool(name="ps_q", bufs=2, space=bass.MemorySpace.PSUM) as ps_qp,
            tc.tile_pool(name="ps_s", bufs=2, space=bass.MemorySpace.PSUM) as ps_sp,
            tc.tile_pool(name="ps_o", bufs=2, space=bass.MemorySpace.PSUM) as ps_op,
        ):
            def q_phase(h):
                wq_h = p_wq.tile([128, NDT, 128], bf16, tag="wq")
                nc.sync.dma_start(out=wq_h, in_=wqview[:, h])
                psq = ps_qp.tile([128, CHUNK], fp32, tag="ps_q")
                for dt in range(NDT):
                    nc.tensor.matmul(
                        psq, wq_h[:, dt, :], xt[dt][:, CHUNK:NK],
                        start=(dt == 0), stop=(dt == NDT - 1),
                    )
                qt[h] = p_qt.tile([128, CHUNK], bf16, tag="qt", name=f"qt{h}")
                _rope(qt[h], psq, cos_sb[:, CHUNK:NK], sin_sb[:, CHUNK:NK])

            def sav_phase(h):
                # S^T per key tile s (wide moving operand), exp, mask
                es = []
                for s in range(NKT):
                    qlo, qhi = qrange(s)
                    w = qhi - qlo
                    pss = ps_sp.tile([128, 512], fp32, tag="ps_s")
                    nc.tensor.matmul(
                        pss[:, 0:w], kt[:, s * 128:(s + 1) * 128],
                        qt[h][:, qlo:qhi], start=True, stop=True,
                    )
                    es_s = p_es.tile([128, 512], bf16, tag="es")
                    nc.scalar.activation(es_s[:, 0:w], pss[:, 0:w], AF.Exp, scale=SCALE)
                    if s <= 3:
                        nc.vector.tensor_mul(
                            es_s[:, w - 128:w], es_s[:, w - 128:w], msk_sb[:, 0:128]
                        )
                    else:
                        nc.vector.tensor_mul(
                            es_s[:, 0:128], es_s[:, 0:128], msk_sb[:, 128:256]
                        )
                    es.append(es_s)
                esbuf[h] = es

                # AV: es stationary, V+ones moving; denominator in col HD
                ob = p_od.tile([128, NQT, HD + 1], fp32, tag="obf")
                for t in range(NQT):
                    otp = ps_op.tile([128, HD + 1], fp32, tag="ps_o")
                    for sig in range(5):
                        s = t + sig
                        off = (t - max(0, s - 4)) * 128
                        nc.tensor.matmul(
                            otp, es[s][:, off:off + 128], v_ext[s],
                            start=(sig == 0), stop=(sig == 4),
                        )
                    nc.vector.tensor_copy(ob[:, t, :], otp)
                obf[h] = ob
                den = p_od.tile([128, NQT], fp32, tag="den")
                nc.vector.tensor_sub(den, ob[:, :, HD], cvec_sb)
                rc = p_od.tile([128, NQT], fp32, tag="rcp")
                nc.vector.reciprocal_approx_fast(rc, den)
                rcp[h] = rc

            def t_phase(h):
                # normalize + transpose back to [hd, q]
                for t in range(NQT):
                    obn = p_tmp.tile([128, HD], bf16, tag="obn")
                    nc.vector.tensor_scalar_mul(
                        obn, obf[h][:, t, 0:HD], rcp[h][:, t:t + 1]
                    )
                    trp = ps_sp.tile([128, 128], bf16, tag="tr")
                    nc.tensor.transpose(trp, obn, eye_sb)
                    nc.scalar.copy(otn[h][:, t * 128:(t + 1) * 128], trp)

            q_phase(0)
            for h in range(H):
                if h + 1 < H:
                    q_phase(h + 1)
                sav_phase(h)
                if h >= 1:
                    t_phase(h - 1)
            t_phase(H - 1)

        # ---- o-projection + bias ----
        woview = wo_d[:].rearrange("p (g h m) -> p g h m", g=NOG, h=H)
        with tc.tile_pool(name="ps_out", bufs=4, space=bass.MemorySpace.PSUM) as ps_outp:
            for g in range(NOG):
                wo_g = p_wo.tile([128, H, 512], bf16, tag="wo")
                nc.sync.dma_start(out=wo_g, in_=woview[:, g])
                for t in range(NQT):
                    pso = ps_outp.tile([128, 512], fp32, tag="ps_out")
                    for h in range(H):
                        nc.tensor.matmul(
                            pso, otn[h][:, t * 128:(t + 1) * 128], wo_g[:, h, :],
                            start=(h == 0), stop=(h == H - 1),
                        )
                    ob = p_ob.tile([128, 512], fp32, tag="ob")
                    nc.vector.tensor_add(ob, pso, bias_sb[:, g * 512:(g + 1) * 512])
                    nc.sync.dma_start(
                        out=out_d[t * 128:(t + 1) * 128, g * 512:(g + 1) * 512],
                        in_=ob,
                    )

    nc.compile()
    return nc


def _get_program():
    global _PROGRAM
    if _PROGRAM is None:
        _PROGRAM = _build_program()
    return _PROGRAM


def _make_in_maps(x, Wq, Wk, Wv, Wo, bo):
    # weights pre-permuted so each device DMA reads contiguous runs
    WqP = np.ascontiguousarray(
        Wq.reshape(NDT, 128, H, 128).transpose(1, 2, 0, 3).reshape(128, H * NDT * 128)
    ).astype(BF16)
    WkP = np.ascontiguousarray(
        Wk.reshape(NDT, 128, HD).transpose(1, 0, 2).reshape(128, NDT * HD)
    ).astype(BF16)
    WvP = np.ascontiguousarray(
        Wv.reshape(NDT, 128, HD).transpose(1, 0, 2).reshape(128, NDT * HD)
    ).astype(BF16)
    WoP = np.ascontiguousarray(
        Wo.reshape(H, 128, NOG, 512).transpose(1, 2, 0, 3).reshape(128, NOG * H * 512)
    ).astype(BF16)
    bo_f = np.ascontiguousarray(bo.astype(np.float32).reshape(1, D))

    inv_freq = np.exp(
        -np.log(np.float32(ROPE_BASE))
        * (np.arange(0, ROPE_DIMS, 2, dtype=np.float32) / np.float32(ROPE_DIMS))
    ).astype(np.float32)

    # universal masks in [k, q] orientation
    kj = np.arange(128, dtype=np.float32)[:, None]
    qi = np.arange(128, dtype=np.float32)[None, :]
    m_far = (qi < kj).astype(np.float32)    # s<=3 far edge: keep iff qi < kj
    m_diag = (qi >= kj).astype(np.float32)  # s>=4 diagonal: keep iff qi >= kj
    maskT = np.ascontiguousarray(
        np.concatenate([m_far, m_diag], axis=1).astype(BF16)
    )
    eye = np.ascontiguousarray(np.eye(128, dtype=np.float32).astype(BF16))

    in_maps = []
    for c in range(8):
        b, g = divmod(c, 4)
        k_start = 512 * g - 512
        xs = np.zeros((NK, D), np.float32)
        lo = max(0, k_start)
        xs[lo - k_start:] = x[b, lo:k_start + NK]
        xT = np.ascontiguousarray(xs.T).astype(BF16)

        pos = (k_start + np.arange(NK)).astype(np.float32)
        theta = pos[None, :] * inv_freq[:, None]          # [32, NK]
        cos2 = np.ascontiguousarray(
            np.concatenate([np.cos(theta)] * 2, axis=0).astype(F16))
        sin2 = np.ascontiguousarray(
            np.concatenate([-np.sin(theta), np.sin(theta)], axis=0).astype(F16))

        # leak counts: zero-padded halo keys contribute exp(0)=1 to the
        # denominator wherever the universal masks keep them
        cvec = np.zeros((128, NQT), np.float32)
        if g == 0:
            qi1 = np.arange(128, dtype=np.float32)
            for t in range(NQT):
                for sig in range(5):
                    s = t + sig
                    if s >= 4:
                        continue  # real keys
                    if sig == 0:
                        cvec[:, t] += 127.0 - qi1
                    else:
                        cvec[:, t] += 128.0

        in_maps.append({
            "xT": xT, "WqP": WqP, "WkP": WkP, "WvP": WvP, "WoP": WoP,
            "bo": bo_f, "cosT": cos2, "sinT": sin2, "maskT": maskT,
            "cvec": np.ascontiguousarray(cvec), "eye": eye,
        })
    return in_maps


def _unshard(results):
    out = np.zeros((B, L, D), np.float32)
    for c in range(8):
        b, g = divmod(c, 4)
        out[b, CHUNK * g:CHUNK * (g + 1)] = results[c]["out"]
    return out


def kernel(x, Wq, Wk, Wv, Wo, bo):
    from concourse.bass_utils import run_bass_kernel_spmd

    nc = _get_program()
    in_maps = _make_in_maps(x, Wq, Wk, Wv, Wo, bo)
    res = run_bass_kernel_spmd(nc, in_maps, core_ids=list(range(8)))
    return _unshard(res.results)


# revision 8
# speedup vs baseline: 1.4791x; 1.0024x over previous
"""Local sliding-window attention block (MQA + partial RoPE) on 8 TRN2 cores.

Sharding: 2 batches x 4 sequence chunks of 512 queries each. Each core
computes q/k/v projections for its chunk (keys include a 512-token halo),
windowed attention (window=512, causal), and the o-projection for its own
query rows — the host-side unshard is a pure concatenation.

v2 layout/engine plan:
  - All weights pre-permuted on the host so every DMA is contiguous per
    partition (4-16 KB packets instead of 256 B).
  - S^T tiles computed key-tile-major: one matmul per key tile s with the
    contiguous valid query range as the moving operand (8 wide matmuls per
    head instead of 20 narrow ones).
  - Masks are two universal 128x128 tiles (far edge / diagonal); key-halo
    zero-padding leaks exp(0)=1 terms into the softmax denominator, which
    are removed by a host-precomputed per-query leak count (cvec).
  - AV uses es (P^T tile) as the stationary operand and V with an appended
    ones-column as the moving operand: out[q, 0:128] = attention numerator,
    out[q, 128] = softmax denominator — no GPSIMD partition reduce, no
    strided sig-reduce, and the reciprocal shrinks to [128, 4] per head
    (reciprocal_approx_fast).
  - Normalized O tiles are transposed back to [hd, q] with PE transpose-mode
    matmuls to feed the o-projection as stationary operands.
  - Head loop is software-pipelined: Q-proj of head h+1 and the O-transposes
    of head h-1 are interleaved so the tensor queue never waits on the
    RoPE/softmax chains.
"""

import numpy as np
import ml_dtypes

BF16 = ml_dtypes.bfloat16
F16 = np.float16

B, L, D = 2, 2048, 2048
H, HD = 16, 128
ROPE_DIMS, HALF = 64, 32
WINDOW = 512
ROPE_BASE = 10000.0
SCALE = HD ** -0.5

CHUNK = 512            # queries per core
NK = 1024              # keys (incl. halo) per core
NQT = CHUNK // 128     # 4 local query tiles
NKT = NK // 128        # 8 local key tiles
NDT = D // 128         # 16 contraction tiles over embedding dim
NOG = D // 512         # 4 o-proj column groups

_PROGRAM = None


def _build_program():
    from contextlib import ExitStack
    import concourse.bass as bass
    import concourse.mybir as mybir
    import concourse.tile as tile
    from concourse import bacc

    fp32 = mybir.dt.float32
    bf16 = mybir.dt.bfloat16
    f16 = mybir.dt.float16
    AF = mybir.ActivationFunctionType

    nc = bacc.Bacc(None, target_bir_lowering=False)

    xT_d = nc.dram_tensor("xT", [D, NK], bf16, kind="ExternalInput")
    wq_d = nc.dram_tensor("WqP", [128, H * NDT * 128], bf16, kind="ExternalInput")
    wk_d = nc.dram_tensor("WkP", [128, NDT * HD], bf16, kind="ExternalInput")
    wv_d = nc.dram_tensor("WvP", [128, NDT * HD], bf16, kind="ExternalInput")
    wo_d = nc.dram_tensor("WoP", [128, NOG * H * 512], bf16, kind="ExternalInput")
    bo_d = nc.dram_tensor("bo", [1, D], fp32, kind="ExternalInput")
    cos_d = nc.dram_tensor("cosT", [ROPE_DIMS, NK], f16, kind="ExternalInput")
    sin_d = nc.dram_tensor("sinT", [ROPE_DIMS, NK], f16, kind="ExternalInput")
    msk_d = nc.dram_tensor("maskT", [128, 256], bf16, kind="ExternalInput")
    cvec_d = nc.dram_tensor("cvec", [128, NQT], fp32, kind="ExternalInput")
    eye_d = nc.dram_tensor("eye", [128, 128], bf16, kind="ExternalInput")
    out_d = nc.dram_tensor("out", [CHUNK, D], fp32, kind="ExternalOutput")

    def qrange(s):
        # query tiles t covered by key tile s: t in [max(0,s-4), min(3,s)]
        tlo, thi = max(0, s - 4), min(NQT - 1, s)
        return tlo * 128, (thi + 1) * 128

    with tile.TileContext(nc) as tc, ExitStack() as ctx:
        p_const = ctx.enter_context(tc.tile_pool(name="const", bufs=1))
        p_xt = ctx.enter_context(tc.tile_pool(name="xt", bufs=1))
        p_kv = ctx.enter_context(tc.tile_pool(name="kv", bufs=1))
        p_wq = ctx.enter_context(tc.tile_pool(name="wq", bufs=3))
        p_qt = ctx.enter_context(tc.tile_pool(name="qt", bufs=3))
        p_es = ctx.enter_context(tc.tile_pool(name="es", bufs=12))
        p_tmp = ctx.enter_context(tc.tile_pool(name="tmp", bufs=3))
        p_otn = ctx.enter_context(tc.tile_pool(name="otn", bufs=1))
        p_od = ctx.enter_context(tc.tile_pool(name="od", bufs=3))
        p_wo = ctx.enter_context(tc.tile_pool(name="wo", bufs=2))
        p_ob = ctx.enter_context(tc.tile_pool(name="ob", bufs=4))

        def _rope(out_bf, ps, cos2, sin2m):
            """out[0:64] = rotary(ps[0:64]); out[64:128] = ps[64:128].

            cos2 rows [0:32]==[32:64]==cos(theta); sin2m rows [0:32]==-sin,
            [32:64]==+sin. The half-swap goes through two partition-shifting
            SBUF->SBUF DMAs; everything else is partition-aligned math.
            """
            n = cos2.shape[-1]
            sb64 = p_tmp.tile([ROPE_DIMS, n], f16, tag="rope_sb64")
            nc.scalar.copy(sb64, ps[0:ROPE_DIMS])
            ss = p_tmp.tile([ROPE_DIMS, n], f16, tag="rope_ss")
            nc.gpsimd.dma_start(out=ss[0:HALF], in_=sb64[HALF:ROPE_DIMS])
            nc.gpsimd.dma_start(out=ss[HALF:ROPE_DIMS], in_=sb64[0:HALF])
            mcos = p_tmp.tile([ROPE_DIMS, n], f16, tag="rope_mcos")
            nc.vector.tensor_mul(mcos, sb64, cos2)
            nc.vector.tensor_mul(ss, ss, sin2m)
            nc.vector.tensor_add(out_bf[0:ROPE_DIMS], mcos, ss)
            nc.scalar.copy(out_bf[ROPE_DIMS:HD], ps[ROPE_DIMS:HD])

        # ---- persistent loads (all contiguous per partition) ----
        # Small constants go first on the sync queue; the 4 MiB xT stream
        # rides the scalar engine's HWDGE queue so the first K-proj matmul
        # isn't stuck behind it.
        xview = xT_d[:].rearrange("(n p) m -> n p m", p=128)

        wk_sb = p_const.tile([128, NDT, HD], bf16, tag="wk")
        nc.sync.dma_start(
            out=wk_sb, in_=wk_d[:].rearrange("p (n m) -> p n m", n=NDT)
        )
        wv_sb = p_const.tile([128, NDT, HD], bf16, tag="wv")
        nc.sync.dma_start(
            out=wv_sb, in_=wv_d[:].rearrange("p (n m) -> p n m", n=NDT)
        )

        cos_sb = p_const.tile([ROPE_DIMS, NK], f16, tag="cos")
        nc.sync.dma_start(out=cos_sb, in_=cos_d[:])
        sin_sb = p_const.tile([ROPE_DIMS, NK], f16, tag="sin")
        nc.sync.dma_start(out=sin_sb, in_=sin_d[:])

        msk_sb = p_const.tile([128, 256], bf16, tag="msk")
        nc.sync.dma_start(out=msk_sb, in_=msk_d[:])
        cvec_sb = p_const.tile([128, NQT], fp32, tag="cvec")
        nc.sync.dma_start(out=cvec_sb, in_=cvec_d[:])
        eye_sb = p_const.tile([128, 128], bf16, tag="eye")
        nc.sync.dma_start(out=eye_sb, in_=eye_d[:])

        bias_sb = p_const.tile([128, D], fp32, tag="bias")
        nc.sync.dma_start(
            out=bias_sb, in_=bass.AP(tensor=bo_d, offset=0, ap=[[0, 128], [1, D]])
        )

        xt = []
        for i in range(NDT):
            t_ = p_xt.tile([128, NK], bf16, tag=f"xt{i}")
            nc.scalar.dma_start(out=t_, in_=xview[i])
            xt.append(t_)

        # ---- K^T (RoPE'd) and V (+ones col) projections ----
        kt = p_kv.tile([128, NK], bf16, tag="kt")
        v_ext = []
        for s in range(NKT):
            t_ = p_kv.tile([128, HD + 1], bf16, tag=f"v{s}")
            nc.vector.memset(t_[:, HD:HD + 1], 1.0)
            v_ext.append(t_)

        with tc.tile_pool(name="ps_kv", bufs=2, space=bass.MemorySpace.PSUM) as ps_kv:
            for nh in range(NK // 512):
                ps = ps_kv.tile([128, 512], fp32, tag="ps_kv")
                cols = slice(nh * 512, (nh + 1) * 512)
                for dt in range(NDT):
                    nc.tensor.matmul(
                        ps, wk_sb[:, dt, :], xt[dt][:, cols],
                        start=(dt == 0), stop=(dt == NDT - 1),
                    )
                _rope(kt[:, cols], ps, cos_sb[:, cols], sin_sb[:, cols])

            for s in range(NKT):
                psv = ps_kv.tile([128, HD], fp32, tag="ps_v")
                cols = slice(s * 128, (s + 1) * 128)
                for dt in range(NDT):
                    nc.tensor.matmul(
                        psv, xt[dt][:, cols], wv_sb[:, dt, :],
                        start=(dt == 0), stop=(dt == NDT - 1),
                    )
                nc.scalar.copy(v_ext[s][:, 0:HD], psv)

        # ---- per-head attention (software-pipelined) ----
        otn = []
        for h in range(H):
            t_ = p_otn.tile([128, CHUNK], bf16, tag=f"otn{h}")
            otn.append(t_)

        wqview = wq_d[:].rearrange("p (h n m) -> p h n m", h=H, n=NDT)
        qt = [None] * H        # rope'd Q^T per head
        esbuf = [None] * H     # 8 es tiles per head
        obf = [None] * H       # [q, t, hd+1] fp32 numerators + denominators
        rcp = [None] * H       # [q, t] reciprocal denominators

        with (
            tc.tile_pool(name="ps_q", bufs=2, space=bass.MemorySpace.PSUM) as ps_qp,
            tc.tile_pool(name="ps_s", bufs=2, space=bass.MemorySpace.PSUM) as ps_sp,
            tc.tile_pool(name="ps_o", bufs=2, space=bass.MemorySpace.PSUM) as ps_op,
        ):
            def q_phase(h):
                wq_h = p_wq.tile([128, NDT, 128], bf16, tag="wq")
                nc.sync.dma_start(out=wq_h, in_=wqview[:, h])
                psq = ps_qp.tile([128, CHUNK], fp32, tag="ps_q")
                for dt in range(NDT):
                    nc.tensor.matmul(
                        psq, wq_h[:, dt, :], xt[dt][:, CHUNK:NK],
                        start=(dt == 0), stop=(dt == NDT - 1),
                    )
                qt[h] = p_qt.tile([128, CHUNK], bf16, tag="qt", name=f"qt{h}")
                _rope(qt[h], psq, cos_sb[:, CHUNK:NK], sin_sb[:, CHUNK:NK])

            def sav_phase(h):
                # S^T per key tile s (wide moving operand), exp, mask
                es = []
                for s in range(NKT):
                    qlo, qhi = qrange(s)
                    w = qhi - qlo
                    pss = ps_sp.tile([128, 512], fp32, tag="ps_s")
                    nc.tensor.matmul(
                        pss[:, 0:w], kt[:, s * 128:(s + 1) * 128],
                        qt[h][:, qlo:qhi], start=True, stop=True,
                    )
                    es_s = p_es.tile([128, 512], bf16, tag="es")
                    nc.scalar.activation(es_s[:, 0:w], pss[:, 0:w], AF.Exp, scale=SCALE)
                    if s <= 3:
                        nc.vector.tensor_mul(
                            es_s[:, w - 128:w], es_s[:, w - 128:w], msk_sb[:, 0:128]
                        )
                    else:
                        nc.vector.tensor_mul(
                            es_s[:, 0:128], es_s[:, 0:128], msk_sb[:, 128:256]
                        )
                    es.append(es_s)
                esbuf[h] = es

                # AV: es stationary, V+ones moving; denominator in col HD
                ob = p_od.tile([128, NQT, HD + 1], fp32, tag="obf")
                for t in range(NQT):
                    otp = ps_op.tile([128, HD + 1], fp32, tag="ps_o")
                    for sig in range(5):
                        s = t + sig
                        off = (t - max(0, s - 4)) * 128
                        nc.tensor.matmul(
                            otp, es[s][:, off:off + 128], v_ext[s],
                            start=(sig == 0), stop=(sig == 4),
                        )
                    nc.vector.tensor_copy(ob[:, t, :], otp)
                obf[h] = ob
                den = p_od.tile([128, NQT], fp32, tag="den")
                nc.vector.tensor_sub(den, ob[:, :, HD], cvec_sb)
                rc = p_od.tile([128, NQT], fp32, tag="rcp")
                nc.vector.reciprocal_approx_fast(rc, den)
                rcp[h] = rc

            def t_phase(h):
                # normalize + transpose back to [hd, q]
                for t in range(NQT):
                    obn = p_tmp.tile([128, HD], bf16, tag="obn")
                    nc.vector.tensor_scalar_mul(
                        obn, obf[h][:, t, 0:HD], rcp[h][:, t:t + 1]
                    )
                    trp = ps_sp.tile([128, 128], bf16, tag="tr")
                    nc.tensor.transpose(trp, obn, eye_sb)
                    nc.scalar.copy(otn[h][:, t * 128:(t + 1) * 128], trp)

            q_phase(0)
            for h in range(H):
                if h + 1 < H:
                    q_phase(h + 1)
                sav_phase(h)
                if h >= 1:
                    t_phase(h - 1)
            t_phase(H - 1)

        # ---- o-projection + bias ----
        woview = wo_d[:].rearrange("p (g h m) -> p g h m", g=NOG, h=H)
        with tc.tile_pool(name="ps_out", bufs=4, space=bass.MemorySpace.PSUM) as ps_outp:
            for g in range(NOG):
                wo_g = p_wo.tile([128, H, 512], bf16, tag="wo")
                nc.sync.dma_start(out=wo_g, in_=woview[:, g])
                for t in range(NQT):
                    pso = ps_outp.tile([128, 512], fp32, tag="ps_out")
                    for h in range(H):
                        nc.tensor.matmul(
                            pso, otn[h][:, t * 128:(t + 1) * 128], wo_g[:, h, :],
                            start=(h == 0), stop=(h == H - 1),
                        )
                    ob = p_ob.tile([128, 512], fp32, tag="ob")
                    nc.vector.tensor_add(ob, pso, bias_sb[:, g * 512:(g + 1) * 512])
                    nc.scalar.dma_start(
                        out=out_d[t * 128:(t + 1) * 128, g * 512:(g + 1) * 512],
                        in_=ob,
                    )

    nc.compile()
    return nc


def _get_program():
    global _PROGRAM
    if _PROGRAM is None:
        _PROGRAM = _build_program()
    return _PROGRAM


def _make_in_maps(x, Wq, Wk, Wv, Wo, bo):
    # weights pre-permuted so each device DMA reads contiguous runs
    WqP = np.ascontiguousarray(
        Wq.reshape(NDT, 128, H, 128).transpose(1, 2, 0, 3).reshape(128, H * NDT * 128)
    ).astype(BF16)
    WkP = np.ascontiguousarray(
        Wk.reshape(NDT, 128, HD).transpose(1, 0, 2).reshape(128, NDT * HD)
    ).astype(BF16)
    WvP = np.ascontiguousarray(
        Wv.reshape(NDT, 128, HD).transpose(1, 0, 2).reshape(128, NDT * HD)
    ).astype(BF16)
    WoP = np.ascontiguousarray(
        Wo.reshape(H, 128, NOG, 512).transpose(1, 2, 0, 3).reshape(128, NOG * H * 512)
    ).astype(BF16)
    bo_f = np.ascontiguousarray(bo.astype(np.float32).reshape(1, D))

    inv_freq = np.exp(
        -np.log(np.float32(ROPE_BASE))
        * (np.arange(0, ROPE_DIMS, 2, dtype=np.float32) / np.float32(ROPE_DIMS))
    ).astype(np.float32)

    # universal masks in [k, q] orientation
    kj = np.arange(128, dtype=np.float32)[:, None]
    qi = np.arange(128, dtype=np.float32)[None, :]
    m_far = (qi < kj).astype(np.float32)    # s<=3 far edge: keep iff qi < kj
    m_diag = (qi >= kj).astype(np.float32)  # s>=4 diagonal: keep iff qi >= kj
    maskT = np.ascontiguousarray(
        np.concatenate([m_far, m_diag], axis=1).astype(BF16)
    )
    eye = np.ascontiguousarray(np.eye(128, dtype=np.float32).astype(BF16))

    in_maps = []
    for c in range(8):
        b, g = divmod(c, 4)
        k_start = 512 * g - 512
        xs = np.zeros((NK, D), np.float32)
        lo = max(0, k_start)
        xs[lo - k_start:] = x[b, lo:k_start + NK]
        xT = np.ascontiguousarray(xs.T).astype(BF16)

        pos = (k_start + np.arange(NK)).astype(np.float32)
        theta = pos[None, :] * inv_freq[:, None]          # [32, NK]
        cos2 = np.ascontiguousarray(
            np.concatenate([np.cos(theta)] * 2, axis=0).astype(F16))
        sin2 = np.ascontiguousarray(
            np.concatenate([-np.sin(theta), np.sin(theta)], axis=0).astype(F16))

        # leak counts: zero-padded halo keys contribute exp(0)=1 to the
        # denominator wherever the universal masks keep them
        cvec = np.zeros((128, NQT), np.float32)
        if g == 0:
            qi1 = np.arange(128, dtype=np.float32)
            for t in range(NQT):
                for sig in range(5):
                    s = t + sig
                    if s >= 4:
                        continue  # real keys
                    if sig == 0:
                        cvec[:, t] += 127.0 - qi1
                    else:
                        cvec[:, t] += 128.0

        in_maps.append({
            "xT": xT, "WqP": WqP, "WkP": WkP, "WvP": WvP, "WoP": WoP,
            "bo": bo_f, "cosT": cos2, "sinT": sin2, "maskT": maskT,
            "cvec": np.ascontiguousarray(cvec), "eye": eye,
        })
    return in_maps


def _unshard(results):
    out = np.zeros((B, L, D), np.float32)
    for c in range(8):
        b, g = divmod(c, 4)
        out[b, CHUNK * g:CHUNK * (g + 1)] = results[c]["out"]
    return out


def kernel(x, Wq, Wk, Wv, Wo, bo):
    from concourse.bass_utils import run_bass_kernel_spmd

    nc = _get_program()
    in_maps = _make_in_maps(x, Wq, Wk, Wv, Wo, bo)
    res = run_bass_kernel_spmd(nc, in_maps, core_ids=list(range(8)))
    return _unshard(res.results)
